# revision 58
# baseline (speedup 1.0000x reference)
"""Trainium2 Bass kernel for a 6-layer dense transformer (patch-embed ->
6x(MHA+FFN) -> token-predictor), sharded across 8 NeuronCores.

Sharding: 4096 tokens (B=4 x N=1024) split 8 ways: core c owns batch c//2,
token half c%2 (512 tokens). Per layer, the pair exchanges the fp8 LN1
output y (feature-major) via a ReduceScatter(add) against a zeroed peer
slot (cheaper than AllGather of K/V); each core computes K/V for both
halves from its local y and the received peer y.

All GEMMs run in fp8(e4m3) with the DoubleRow perf mode (two 128-deep
k-subtiles per instruction). Weights are pre-scaled on the host (x32 or
x256) so fp8 sees well-ranged values; descales fold into activation
scale factors / residual adds. Biases ride in a padded 4th k-tile
(ones-row in the activation, bias row in the weight). Softmax exp and
GELU run on the Activation engine from 2-bank PSUM tiles (pair-merged);
LayerNorm rstd is computed on DVE via quake-rsqrt + 2 Newton steps so
the Act engine only ever loads the Exp and Gelu tables.
"""
import sys
import math

sys.path.insert(0, '/opt/trn_rl_repo')

import numpy as np
import ml_dtypes

B, NTOK, V, D, H, E, F, L = 4, 1024, 8192, 384, 6, 64, 1536, 6
NCORES = 8
T = NTOK * B // NCORES          # 512 tokens per core
TP, DP, FP, VP = T // 128, D // 128, F // 128, V // 128
KT = DP + 1                     # 3 feature k-tiles + bias/pad tile
F2T = FP + 2                    # 12 FFN2 k-tiles + bias/pad pair
EP = E + 1                      # head dim + denominator ones-column
SW = 32.0                       # weight fp8 pre-scale
SQ = 256.0                      # Wq pre-scale (includes E^-0.5 headroom)
SP32 = math.sqrt(32.0)          # unembed split scale
CCSZ = DP * 128 * T             # fp8 bytes of one y half (196608)


def _positional_encoding(n, d):
    position = np.arange(n)[:, None].astype(np.float32)
    div_term = np.exp(np.arange(0, d, 2).astype(np.float32)
                      * (-math.log(10000.0) / d))
    pe = np.zeros((n, d), dtype=np.float32)
    pe[:, 0::2] = np.sin(position * div_term)
    pe[:, 1::2] = np.cos(position * div_term)
    return pe


def build_nc():
    import concourse.bass as bass
    import concourse.mybir as mybir
    import concourse.tile as tile
    from concourse import bacc
    from concourse.bass import ts, ds

    F32 = mybir.dt.float32
    BF16 = mybir.dt.bfloat16
    FP8 = mybir.dt.float8e4
    I32 = mybir.dt.int32
    AF = mybir.ActivationFunctionType
    OP = mybir.AluOpType
    DR = mybir.MatmulPerfMode.DoubleRow

    nc = bacc.Bacc("TRN2", target_bir_lowering=False, debug=False,
                   num_devices=NCORES, num_swdge_queues=4)

    xT8 = nc.dram_tensor("xT8", [128, VP * T], FP8, kind="ExternalInput")
    xT8l = nc.dram_tensor("xT8l", [128, VP * T], FP8, kind="ExternalInput")
    peb = nc.dram_tensor("peb", [T, D], F32, kind="ExternalInput")
    eye32 = nc.dram_tensor("eye32", [128, 128], F32, kind="ExternalInput")
    slotoff = nc.dram_tensor("slotoff", [1, 1], I32, kind="ExternalInput")
    wemb8 = nc.dram_tensor("wemb8", [128, VP * D], FP8, kind="ExternalInput")
    wembl = nc.dram_tensor("wembl", [128, VP * D], FP8, kind="ExternalInput")
    wqkvo8 = nc.dram_tensor("wqkvo8", [L * 4 * 128, KT * D], FP8,
                            kind="ExternalInput")
    w18 = nc.dram_tensor("w18", [L * 128, KT * F], FP8, kind="ExternalInput")
    w28 = nc.dram_tensor("w28", [L * 128, F2T * D], FP8, kind="ExternalInput")
    wp8 = nc.dram_tensor("wp8", [128, KT * V], FP8, kind="ExternalInput")
    wp8l = nc.dram_tensor("wp8l", [128, KT * V], FP8, kind="ExternalInput")
    logits16 = nc.dram_tensor("logits16", [T, V], BF16, kind="ExternalOutput")
    import os
    dbg_stage = int(os.environ.get("KDBG", "-1"))
    dbg = None
    if dbg_stage >= 0:
        dbg = nc.dram_tensor("dbg", [128, 2048], mybir.dt.float32,
                             kind="ExternalOutput")

    RG = [[0, 1], [2, 3], [4, 5], [6, 7]]

    with tile.TileContext(nc) as tc:
        import contextlib
        ctx = contextlib.ExitStack()
        singles = ctx.enter_context(tc.tile_pool(name="singles", bufs=1))

        eye16 = singles.tile([128, 128], BF16, name="eye16", tag="eye16")
        nc.gpsimd.dma_start(out=eye16[:], in_=eye32.ap())
        zst8 = singles.tile([1, 128], FP8, name="zst8", tag="zst8")
        nc.vector.memset(zst8[:], 0.0)
        zmv8 = singles.tile([1, 256], FP8, name="zmv8", tag="zmv8")
        nc.vector.memset(zmv8[:], 0.0)
        ones_b16 = singles.tile([1, E], BF16, name="ones_b16", tag="ones_b16")
        nc.vector.memset(ones_b16[:], 1.0 / SW)
        peb_sb = singles.tile([128, TP, D], F32, name="peb_sb", tag="peb_sb")
        nc.sync.dma_start(peb_sb[:],
                          peb.ap().rearrange("(m p) d -> p m d", p=128))
        resid = singles.tile([128, TP, D], F32, name="resid", tag="resid")
        soff_sb = singles.tile([1, 1], I32, name="soff_sb", tag="soff_sb")
        nc.sync.dma_start(soff_sb[:], slotoff.ap())
        soff_reg = nc.sync.alloc_register("soff_reg")
        nc.sync.reg_load(soff_reg, soff_sb[0:1, 0:1])
        soff = nc.sync.snap(soff_reg, donate=True, min_val=0, max_val=CCSZ)

        # double-buffered (by layer parity) persistent activation tiles with
        # preset pad tiles (ones-row at kt=KT-1 for weight-embedded biases)
        def padded_fm(nm, n_kt, one_tiles):
            bufs = []
            for par in range(2):
                t_ = singles.tile([128, n_kt, T], FP8, name=f"{nm}{par}",
                                  tag=f"{nm}{par}")
                for it in one_tiles:
                    nc.vector.memset(t_[:, it, :], 0.0)
                    nc.vector.memset(t_[0:1, it, :], 1.0)
                bufs.append(t_)
            return bufs

        y_fm2 = padded_fm("y_fm", KT, [KT - 1])
        yrem2 = padded_fm("yrem", KT, [KT - 1])
        y2_fm2 = padded_fm("y2fm", KT, [KT - 1])
        o_fm2 = padded_fm("o_fm", KT, [KT - 1])
        g2 = padded_fm("g", F2T, [FP])     # tile 12 = bias ones-row, 13 zero
        for par in range(2):
            nc.vector.memset(g2[par][:, FP + 1, :], 0.0)

        # persistent collective staging (parity-alternating); peer halves are
        # zeroed once here and only the own half is rewritten per layer
        ccin2, ccout2 = [], []
        zz = singles.tile([128, 2 * CCSZ // 128], FP8, name="zz", tag="zz")
        nc.vector.memset(zz[:], 0.0)
        for par in range(2):
            ci = singles.tile([2 * CCSZ], FP8, name=f"ccin{par}",
                              tag=f"ccin{par}", space="DRAM")
            nc.sync.dma_start(ci[:].rearrange("(p n) -> p n", p=128), zz[:])
            ccin2.append(ci)
            co = singles.tile([CCSZ], FP8, name=f"ccout{par}",
                              tag=f"ccout{par}", space="DRAM")
            ccout2.append(co)

        kfm2, qfm2, vful2 = [], [], []
        for par in range(2):
            k_ = singles.tile([128, DP, 2, T], FP8, name=f"kfm{par}",
                              tag=f"kfm{par}")
            kfm2.append(k_)
            q_ = singles.tile([128, DP, T], FP8, name=f"qfm{par}",
                              tag=f"qfm{par}")
            qfm2.append(q_)
            # cols 0:64 = v, cols 64:128 = ones so the AV DoubleRow matmul
            # emits softmax denominators into output partitions 64..127
            v_ = singles.tile([128, 2 * TP, H, 2 * E], FP8, name=f"vful{par}",
                              tag=f"vful{par}")
            nc.vector.memset(v_[:, :, :, E:2 * E], 1.0)
            vful2.append(v_)

        # ---------------- EMBED ----------------
        with tc.tile_pool(name="xp", bufs=2) as xp, \
             tc.tile_pool(name="wep", bufs=2) as wep, \
             tc.tile_pool(name="embps", bufs=1, space="PSUM") as embps:
            emb_ps = [embps.tile([128, D], F32, name=f"embp{mt}",
                                 tag=f"embp{mt}") for mt in range(TP)]
            NCH = 4
            KCH = VP // NCH                   # 16 k-tiles per chunk
            for c in range(NCH):
                x_t = xp.tile([128, KCH, T], FP8, name="x_t", tag="x")
                nc.sync.dma_start(x_t[:], xT8.ap()[:, c * KCH * T:
                                                   (c + 1) * KCH * T]
                                  .rearrange("p (k n) -> p k n", k=KCH))
                x_l = xp.tile([128, KCH, T], FP8, name="x_l", tag="xl")
                nc.sync.dma_start(x_l[:], xT8l.ap()[:, c * KCH * T:
                                                    (c + 1) * KCH * T]
                                  .rearrange("p (k n) -> p k n", k=KCH))
                w_t = wep.tile([128, KCH, D], FP8, name="w_t", tag="we")
                nc.sync.dma_start(w_t[:], wemb8.ap()[:, c * KCH * D:
                                                     (c + 1) * KCH * D]
                                  .rearrange("p (k o) -> p k o", k=KCH))
                w_l = wep.tile([128, KCH, D], FP8, name="w_l", tag="wel")
                nc.sync.dma_start(w_l[:], wembl.ap()[:, c * KCH * D:
                                                     (c + 1) * KCH * D]
                                  .rearrange("p (k o) -> p k o", k=KCH))
                for j in range(KCH // 2):
                    for mt in range(TP):
                        sl = ts(mt, 128)
                        kk = slice(2 * j, 2 * j + 2)
                        nc.tensor.matmul(emb_ps[mt][:], x_t[:, kk, sl],
                                         w_t[:, kk, :],
                                         start=(c == 0 and j == 0),
                                         stop=False, perf_mode=DR)
                        nc.tensor.matmul(emb_ps[mt][:], x_l[:, kk, sl],
                                         w_t[:, kk, :], start=False,
                                         stop=False, perf_mode=DR)
                        nc.tensor.matmul(emb_ps[mt][:], x_t[:, kk, sl],
                                         w_l[:, kk, :], start=False,
                                         stop=(c == NCH - 1 and
                                               j == KCH // 2 - 1),
                                         perf_mode=DR)
            for mt in range(TP):
                nc.vector.scalar_tensor_tensor(
                    resid[:, mt, :], emb_ps[mt][:], 1.0 / SW,
                    peb_sb[:, mt, :], op0=OP.mult, op1=OP.add)
        if dbg_stage == 0:
            nc.sync.dma_start(dbg.ap()[:, 0:TP * D],
                              resid[:].rearrange("p m d -> p (m d)"))

        # ---------------- shared helpers ----------------
        smallp = ctx.enter_context(tc.tile_pool(name="smallp", bufs=4))
        # unembed weight pool lives in the whole-kernel scope so its DMAs
        # can prefetch during the last layer (no SBUF-reuse barrier)
        wpp = ctx.enter_context(tc.tile_pool(name="wpp", bufs=2))

        def layernorm(dst_tm, src, rstd_extra_scale=None):
            """token-major LN: sums on the (otherwise idle) Pool engine,
            quake rsqrt + apply on DVE; writes normalized bf16 into dst_tm."""
            mv8 = smallp.tile([128, TP, 2], F32, name="mv8", tag="mv8")
            for mt in range(TP):
                st = smallp.tile([128, 6], F32, name="st", tag="st")
                nc.vector.bn_stats(st[:], src[:, mt, :])
                nc.vector.bn_aggr(mv8[:, mt, :], st[:])
            mean4 = mv8[:, :, 0]
            veps = smallp.tile([128, TP], F32, name="veps", tag="veps")
            nc.vector.tensor_scalar_add(veps[:], mv8[:, :, 1], 1e-5)
            ti = smallp.tile([128, TP], I32, name="ti", tag="ti")
            nc.vector.tensor_scalar(ti[:], veps[:].bitcast(I32), 1,
                                    0xFFFFFFFF, op0=OP.logical_shift_right,
                                    op1=OP.bitwise_xor)
            nc.vector.tensor_scalar_add(ti[:], ti[:], 0x5F375A00)
            y0 = ti[:].bitcast(F32)
            aq = smallp.tile([128, TP], F32, name="aq", tag="aq")
            rstd = smallp.tile([128, TP], F32, name="rstd", tag="rstd")
            nc.vector.tensor_tensor(aq[:], y0, y0, op=OP.mult)
            nc.vector.tensor_tensor(aq[:], aq[:], veps[:], op=OP.mult)
            nc.vector.tensor_scalar(aq[:], aq[:], -0.5, 1.5,
                                    op0=OP.mult, op1=OP.add)
            nc.vector.tensor_tensor(rstd[:], y0, aq[:], op=OP.mult)
            nc.vector.tensor_tensor(aq[:], rstd[:], rstd[:], op=OP.mult)
            nc.vector.tensor_tensor(aq[:], aq[:], veps[:], op=OP.mult)
            nc.vector.tensor_scalar(aq[:], aq[:], -0.5, 1.5,
                                    op0=OP.mult, op1=OP.add)
            nc.vector.tensor_tensor(rstd[:], rstd[:], aq[:], op=OP.mult)
            if rstd_extra_scale is not None:
                nc.vector.tensor_scalar_mul(rstd[:], rstd[:],
                                            rstd_extra_scale)
            for mt in range(TP):
                nc.vector.tensor_scalar(dst_tm[:, mt, :], src[:, mt, :],
                                        mv8[:, mt, 0:1],
                                        rstd[:, mt:mt + 1],
                                        op0=OP.subtract, op1=OP.mult)

        def transpose_to(dst_fm, src_tm, tpool, lo_fm=None):
            """PE-transpose token-major bf16 -> feature-major fp8. All four
            token-tiles of one k-tile share a pre-zeroed PSUM bank
            (accumulate mode), drained by a single DVE copy. lo_fm also
            stores the fp8 quantization residual (error compensation)."""
            for kt in range(DP):
                tp_ = tpool.tile([128, T], BF16, name="tp", tag="mm")
                nc.tensor.matmul(tp_[:].bitcast(F32)[:, 0:T // 2],
                                 zst8[:], zmv8[:, 0:T // 2],
                                 start=True, stop=False,
                                 skip_group_check=True)
                for mt in range(TP):
                    nc.tensor.matmul(tp_[:, ts(mt, 128)],
                                     src_tm[:, mt, ts(kt, 128)], eye16[:],
                                     is_transpose=True, start=False,
                                     stop=(mt == TP - 1),
                                     skip_group_check=True)
                dst = dst_fm[:, kt, :]
                nc.vector.tensor_copy(dst, tp_[:])
                if lo_fm is not None:
                    nc.vector.scalar_tensor_tensor(
                        lo_fm[:, kt, :], tp_[:], 1.0, dst,
                        op0=OP.mult, op1=OP.subtract)

        # ---------------- LAYERS ----------------
        with tc.tile_pool(name="wq4", bufs=2) as wq4, \
             tc.tile_pool(name="w1p", bufs=2) as w1p, \
             tc.tile_pool(name="w2p", bufs=2) as w2p, \
             tc.tile_pool(name="ytm", bufs=2) as ytmp, \
             tc.tile_pool(name="pl", bufs=14) as plp, \
             tc.tile_pool(name="pr", bufs=4) as prp, \
             tc.tile_pool(name="rcp", bufs=3) as rcp, \
             tc.tile_pool(name="psM", bufs=2, space="PSUM") as psM, \
             tc.tile_pool(name="psS", bufs=2, space="PSUM") as psS, \
             tc.tile_pool(name="psO", bufs=2, space="PSUM") as psO:

            for l in range(L):
                par = l % 2
                y_fm, yrem, y2_fm = y_fm2[par], yrem2[par], y2_fm2[par]
                o_fm, g_t = o_fm2[par], g2[par]
                k_fm, q_fm, v_ful = kfm2[par], qfm2[par], vful2[par]

                # --- weight loads (fp8, prescaled, bias rows embedded) ---
                wq_sb = wq4.tile([128, KT, D], FP8, name="wq_sb", tag="wq")
                nc.sync.dma_start(wq_sb[:],
                                  wqkvo8.ap()[(4 * l + 0) * 128:
                                              (4 * l + 1) * 128, :]
                                  .rearrange("p (k o) -> p k o", k=KT))
                wk_sb = wq4.tile([128, KT, D], FP8, name="wk_sb", tag="wk")
                nc.sync.dma_start(wk_sb[:],
                                  wqkvo8.ap()[(4 * l + 1) * 128:
                                              (4 * l + 2) * 128, :]
                                  .rearrange("p (k o) -> p k o", k=KT))
                wv_sb = wq4.tile([128, KT, D], FP8, name="wv_sb", tag="wv")
                nc.sync.dma_start(wv_sb[:],
                                  wqkvo8.ap()[(4 * l + 2) * 128:
                                              (4 * l + 3) * 128, :]
                                  .rearrange("p (k o) -> p k o", k=KT))
                wo_sb = wq4.tile([128, KT, D], FP8, name="wo_sb", tag="wo")
                nc.sync.dma_start(wo_sb[:],
                                  wqkvo8.ap()[(4 * l + 3) * 128:
                                              (4 * l + 4) * 128, :]
                                  .rearrange("p (k o) -> p k o", k=KT))
                w1_sb = w1p.tile([128, KT, F], FP8, name="w1_sb", tag="w1")
                nc.sync.dma_start(w1_sb[:],
                                  w18.ap()[l * 128:(l + 1) * 128, :]
                                  .rearrange("p (k o) -> p k o", k=KT))
                w2_sb = w2p.tile([128, F2T, D], FP8, name="w2_sb", tag="w2")
                nc.sync.dma_start(w2_sb[:],
                                  w28.ap()[l * 128:(l + 1) * 128, :]
                                  .rearrange("p (k o) -> p k o", k=F2T))

                # --- LN1 (bf16) + transpose to fp8 feature-major ---
                y_tm = ytmp.tile([128, TP, D], BF16, name="y_tm", tag="y_tm")
                layernorm(y_tm, resid)
                transpose_to(y_fm, y_tm, psM)

                # --- fire the pairwise y exchange ASAP ---
                cc_in, cc_out = ccin2[par], ccout2[par]
                nc.sync.dma_start(
                    cc_in[ds(soff, CCSZ)]
                    .rearrange("(k p n) -> p k n", p=128, n=T),
                    y_fm[:, 0:DP, :])
                nc.gpsimd.collective_compute(
                    "ReduceScatter", OP.add, replica_groups=RG,
                    ins=[cc_in[:].opt()], outs=[cc_out[:].opt()])

                # --- local K/V/Q projections (overlap the collective) ---
                for t in range(DP):
                    psk = psM.tile([128, T], F32, name="psk", tag="mm")
                    for j in range(KT // 2):
                        nc.tensor.matmul(psk[:],
                                         wk_sb[:, 2 * j:2 * j + 2,
                                               ts(t, 128)],
                                         y_fm[:, 2 * j:2 * j + 2, :],
                                         start=(j == 0), stop=(j == 1),
                                         perf_mode=DR)
                    nc.vector.tensor_copy(k_fm[:, t, 0, :], psk[:])
                for mt in range(TP):
                    psv = psM.tile([128, D], F32, name="psv", tag="mm")
                    for j in range(KT // 2):
                        nc.tensor.matmul(psv[:],
                                         y_fm[:, 2 * j:2 * j + 2,
                                              ts(mt, 128)],
                                         wv_sb[:, 2 * j:2 * j + 2, :],
                                         start=(j == 0), stop=(j == 1),
                                         perf_mode=DR)
                    nc.vector.tensor_copy(
                        v_ful[:, mt, :, 0:E],
                        psv[:].rearrange("p (h e) -> p h e", h=H))
                for t in range(DP):
                    psq = psM.tile([128, T], F32, name="psq", tag="mm")
                    for j in range(KT // 2):
                        nc.tensor.matmul(psq[:],
                                         wq_sb[:, 2 * j:2 * j + 2,
                                               ts(t, 128)],
                                         y_fm[:, 2 * j:2 * j + 2, :],
                                         start=(j == 0), stop=(j == 1),
                                         perf_mode=DR)
                    nc.vector.tensor_copy(q_fm[:, t, :], psq[:])

                # --- local-half scores + exp (hides the collective) ---
                p_loc = {}
                for h in range(H):
                    po, pt = (h % 2) * E, h // 2
                    for pj in range(TP // 2):
                        sc = psS.tile([128, 2 * T], F32, name="sc", tag="sc")
                        for u in range(2):
                            m = 2 * pj + u
                            nc.tensor.matmul(sc[:, u * T:(u + 1) * T],
                                             k_fm[po:po + E, pt, 0,
                                                  ts(m, 128)],
                                             q_fm[po:po + E, pt, :],
                                             start=True, stop=True)
                        p_ = plp.tile([128, 2, T], FP8, name="p_", tag="p")
                        nc.scalar.activation(
                            p_[:].rearrange("p a b -> p (a b)"), sc[:],
                            AF.Exp, scale=1.0 / (SQ * SW))
                        p_loc[(h, pj)] = p_

                # receive the peer half; issued on the Act queue *after* the
                # local exps so its collective-wait blocks nothing upstream
                nc.scalar.dma_start(
                    yrem[:, 0:DP, :],
                    cc_out[:].rearrange("(k p n) -> p k n", p=128, n=T))

                # --- remote K/V projections from the received y ---
                for t in range(DP):
                    psk = psM.tile([128, T], F32, name="pskr", tag="mm")
                    for j in range(KT // 2):
                        nc.tensor.matmul(psk[:],
                                         wk_sb[:, 2 * j:2 * j + 2,
                                               ts(t, 128)],
                                         yrem[:, 2 * j:2 * j + 2, :],
                                         start=(j == 0), stop=(j == 1),
                                         perf_mode=DR)
                    nc.vector.tensor_copy(k_fm[:, t, 1, :], psk[:])
                for mt in range(TP):
                    psv = psM.tile([128, D], F32, name="psvr", tag="mm")
                    for j in range(KT // 2):
                        nc.tensor.matmul(psv[:],
                                         yrem[:, 2 * j:2 * j + 2,
                                              ts(mt, 128)],
                                         wv_sb[:, 2 * j:2 * j + 2, :],
                                         start=(j == 0), stop=(j == 1),
                                         perf_mode=DR)
                    nc.vector.tensor_copy(
                        v_ful[:, TP + mt, :, 0:E],
                        psv[:].rearrange("p (h e) -> p h e", h=H))

                # --- remote scores + exp, then AV + normalize per head ---
                for h in range(H):
                    po, pt = (h % 2) * E, h // 2
                    p_rem = []
                    for pj in range(TP // 2):
                        sc = psS.tile([128, 2 * T], F32, name="scr", tag="sc")
                        for u in range(2):
                            m = 2 * pj + u
                            nc.tensor.matmul(sc[:, u * T:(u + 1) * T],
                                             k_fm[po:po + E, pt, 1,
                                                  ts(m, 128)],
                                             q_fm[po:po + E, pt, :],
                                             start=True, stop=True)
                        p_ = prp.tile([128, 2, T], FP8, name="pr_", tag="pr")
                        nc.scalar.activation(
                            p_[:].rearrange("p a b -> p (a b)"), sc[:],
                            AF.Exp, scale=1.0 / (SQ * SW))
                        p_rem.append(p_)
                    o_ps = psO.tile([128, T], F32, name="o_ps", tag="o")
                    for pj in range(TP // 2):
                        nc.tensor.matmul(o_ps[:],
                                         v_ful[:, 2 * pj:2 * pj + 2, h, :],
                                         p_loc[(h, pj)][:],
                                         start=(pj == 0), stop=False,
                                         perf_mode=DR)
                    for pj in range(TP // 2):
                        nc.tensor.matmul(o_ps[:],
                                         v_ful[:, TP + 2 * pj:TP + 2 * pj + 2,
                                               h, :],
                                         p_rem[pj][:],
                                         start=False, stop=(pj == 1),
                                         perf_mode=DR)
                    recip = rcp.tile([1, T], BF16, name="recip", tag="recip")
                    with nc.allow_low_precision(reason="softmax recip"):
                        nc.vector.reciprocal(recip[:], o_ps[E:E + 1, :])
                    bc_ps = psM.tile([E, T], F32, name="bc_ps", tag="mm")
                    nc.tensor.matmul(bc_ps[:], ones_b16[:], recip[:],
                                     start=True, stop=True)
                    bc_sb = rcp.tile([E, T], F32, name="bc_sb", tag="bc")
                    nc.vector.tensor_copy(bc_sb[:], bc_ps[:])
                    nc.vector.tensor_tensor(o_fm[po:po + E, pt, :],
                                            o_ps[0:E, :], bc_sb[:],
                                            op=OP.mult)

                # --- Wo + residual ---
                for mt in range(TP):
                    pso = psM.tile([128, D], F32, name="pso", tag="mm")
                    for j in range(KT // 2):
                        nc.tensor.matmul(pso[:],
                                         o_fm[:, 2 * j:2 * j + 2,
                                              ts(mt, 128)],
                                         wo_sb[:, 2 * j:2 * j + 2, :],
                                         start=(j == 0), stop=(j == 1),
                                         perf_mode=DR)
                    nc.vector.scalar_tensor_tensor(
                        resid[:, mt, :], pso[:], 1.0 / SW, resid[:, mt, :],
                        op0=OP.mult, op1=OP.add)

                if dbg_stage == 100 + l:
                    nc.sync.dma_start(dbg.ap()[:, 0:TP * D],
                                      resid[:].rearrange("p m d -> p (m d)"))
                if dbg_stage == 200 + l:
                    nc.gpsimd.dma_start(
                        out=dbg.ap(),
                        in_=o_fm[:].rearrange("p k n -> p (k n)"))
                if dbg_stage == 300 + l:
                    nc.gpsimd.dma_start(
                        out=dbg.ap(),
                        in_=k_fm[:].rearrange("p k r n -> p (k r n)")
                        [:, 0:2048])

                # --- LN2 + transpose ---
                y2_tm = ytmp.tile([128, TP, D], BF16, name="y2_tm",
                                  tag="y_tm")
                layernorm(y2_tm, resid)
                transpose_to(y2_fm, y2_tm, psM)

                # --- FFN: FFN1 pair -> gelu -> fp8 g; FFN2 DR pairs ---
                for fj in range(FP // 2):
                    psf = psS.tile([128, 2 * T], F32, name="psf", tag="sc")
                    for u in range(2):
                        ft = 2 * fj + u
                        for j in range(KT // 2):
                            nc.tensor.matmul(psf[:, u * T:(u + 1) * T],
                                             w1_sb[:, 2 * j:2 * j + 2,
                                                   ts(ft, 128)],
                                             y2_fm[:, 2 * j:2 * j + 2, :],
                                             start=(j == 0), stop=(j == 1),
                                             perf_mode=DR)
                    nc.scalar.activation(
                        g_t[:, 2 * fj:2 * fj + 2, :]
                        .rearrange("p a b -> p (a b)"),
                        psf[:], AF.Gelu, scale=1.0 / SW)
                for mt in range(TP):
                    psf2 = psM.tile([128, D], F32, name="psf2", tag="mm")
                    for j in range(F2T // 2):
                        nc.tensor.matmul(psf2[:],
                                         g_t[:, 2 * j:2 * j + 2,
                                             ts(mt, 128)],
                                         w2_sb[:, 2 * j:2 * j + 2, :],
                                         start=(j == 0), stop=(j == F2T // 2 - 1),
                                         perf_mode=DR)
                    nc.vector.scalar_tensor_tensor(
                        resid[:, mt, :], psf2[:], 1.0 / SW, resid[:, mt, :],
                        op0=OP.mult, op1=OP.add)
                if dbg_stage == 1 + l:
                    nc.sync.dma_start(dbg.ap()[:, 0:TP * D],
                                      resid[:].rearrange("p m d -> p (m d)"))

        # ---------------- FINAL LN + UNEMBED ----------------
        with tc.tile_pool(name="lntp", bufs=2) as lntp, \
             tc.tile_pool(name="tpp", bufs=2, space="PSUM") as tpp, \
             tc.tile_pool(name="lgp", bufs=4) as lgp, \
             tc.tile_pool(name="psl", bufs=2, space="PSUM") as pslp:
            lnf_tm = lntp.tile([128, TP, D], BF16, name="lnf_tm", tag="lnf")
            layernorm(lnf_tm, resid, rstd_extra_scale=1.0 / SP32)
            lnf_fm = lntp.tile([128, KT, T], FP8, name="lnf_fm", tag="lnffm")
            lnf_lo = lntp.tile([128, KT, T], FP8, name="lnf_lo", tag="lnflo")
            nc.vector.memset(lnf_fm[:, KT - 1, :], 0.0)
            nc.vector.memset(lnf_fm[0:1, KT - 1, :], 1.0)
            nc.vector.memset(lnf_lo[:, KT - 1, :], 0.0)
            transpose_to(lnf_fm, lnf_tm, tpp, lo_fm=lnf_lo)

            NVC = 4                       # vocab chunks of 2048
            VCW = V // NVC
            for c in range(NVC):
                wp_t = wpp.tile([128, KT, VCW], FP8, name="wp_t", tag="wp")
                nc.sync.dma_start(
                    wp_t[:],
                    wp8.ap().rearrange("p (k v) -> p k v", k=KT)
                    [:, :, c * VCW:(c + 1) * VCW])
                wp_l = wpp.tile([128, KT, VCW], FP8, name="wp_l", tag="wpl")
                nc.sync.dma_start(
                    wp_l[:],
                    wp8l.ap().rearrange("p (k v) -> p k v", k=KT)
                    [:, :, c * VCW:(c + 1) * VCW])
                chains = [(lnf_fm, wp_t), (lnf_lo, wp_t), (lnf_fm, wp_l)]
                for hp in range(VCW // (2 * T)):
                    vc0 = c * (VCW // T) + 2 * hp
                    for mt in range(TP):
                        psl = pslp.tile([128, 2 * T], F32, name="psl",
                                        tag="lg")
                        for u in range(2):
                            hsl = slice((2 * hp + u) * T,
                                        (2 * hp + u + 1) * T)
                            for ci, (lf, wp_) in enumerate(chains):
                                for j in range(KT // 2):
                                    nc.tensor.matmul(
                                        psl[:, u * T:(u + 1) * T],
                                        lf[:, 2 * j:2 * j + 2, ts(mt, 128)],
                                        wp_[:, 2 * j:2 * j + 2, hsl],
                                        start=(ci == 0 and j == 0),
                                        stop=(ci == 2 and j == 1),
                                        perf_mode=DR)
                        lg = lgp.tile([128, 2 * T], BF16, name="lg",
                                      tag="lgs")
                        if mt % 2 == 0:
                            nc.vector.tensor_copy(lg[:], psl[:])
                        else:
                            nc.scalar.copy(lg[:], psl[:])
                        nc.sync.dma_start(
                            logits16.ap()[mt * 128:(mt + 1) * 128,
                                          vc0 * T:(vc0 + 2) * T], lg[:])
        ctx.close()

    nc.compile()
    return nc


def _prep_inputs(inputs):
    f8 = ml_dtypes.float8_e4m3
    f = {k: np.asarray(v, dtype=np.float32) for k, v in inputs.items()}
    x, Wemb_, bemb = f["x"], f["Wemb"], f["bemb"]
    scale = E ** -0.5

    def pack_w(w, brow, n_kt, n_out, lo=False):
        """[D_in, n_out] + bias row -> fp8 [128, n_kt*n_out] p-major with
        bias in the first row of the pad k-tile. lo=True also returns the
        fp8 quantization residual (error-compensation term)."""
        wp = np.zeros((n_kt * 128, n_out), np.float32)
        wp[:w.shape[0]] = w
        wp[w.shape[0]] = brow
        wp = wp.reshape(n_kt, 128, n_out).transpose(1, 0, 2) \
               .reshape(128, n_kt * n_out)
        hi = np.ascontiguousarray(wp.astype(f8))
        if not lo:
            return hi
        rem = np.ascontiguousarray((wp - hi.astype(np.float32)).astype(f8))
        return hi, rem

    qkvo = np.empty((L * 4 * 128, KT * D), f8)
    w1p_ = np.empty((L * 128, KT * F), f8)
    w2p_ = np.empty((L * 128, F2T * D), f8)
    for l in range(L):
        g1, b1l = f["ln1_g"][l], f["ln1_b"][l]
        Wq_l = f["Wq"][l].transpose(1, 0, 2).reshape(D, D)
        Wk_l = f["Wk"][l].transpose(1, 0, 2).reshape(D, D)
        Wv_l = f["Wv"][l].transpose(1, 0, 2).reshape(D, D)
        qkvo[(4 * l + 0) * 128:(4 * l + 1) * 128] = pack_w(
            g1[:, None] * Wq_l * (scale * SQ),
            (b1l @ Wq_l + f["bq"][l].reshape(-1)) * (scale * SQ), KT, D)
        qkvo[(4 * l + 1) * 128:(4 * l + 2) * 128] = pack_w(
            g1[:, None] * Wk_l * SW,
            (b1l @ Wk_l + f["bk"][l].reshape(-1)) * SW, KT, D)
        qkvo[(4 * l + 2) * 128:(4 * l + 3) * 128] = pack_w(
            g1[:, None] * Wv_l * SW,
            (b1l @ Wv_l + f["bv"][l].reshape(-1)) * SW, KT, D)
        qkvo[(4 * l + 3) * 128:(4 * l + 4) * 128] = pack_w(
            f["Wo"][l] * SW, f["bo"][l] * SW, KT, D)
        g2l, b2l = f["ln2_g"][l], f["ln2_b"][l]
        w1p_[l * 128:(l + 1) * 128] = pack_w(
            g2l[:, None] * f["W1"][l] * SW,
            (b2l @ f["W1"][l] + f["b1"][l]) * SW, KT, F)
        w2full = np.zeros((F2T * 128, D), np.float32)
        w2full[:F] = f["W2"][l] * SW
        w2full[F] = f["b2"][l] * SW
        w2p_[l * 128:(l + 1) * 128] = np.ascontiguousarray(
            w2full.reshape(F2T, 128, D).transpose(1, 0, 2)
            .reshape(128, F2T * D).astype(f8))

    wemb_s = (Wemb_ * SW).reshape(VP, 128, D).transpose(1, 0, 2) \
        .reshape(128, VP * D)
    wemb8 = np.ascontiguousarray(wemb_s.astype(f8))
    wembl = np.ascontiguousarray(
        (wemb_s - wemb8.astype(np.float32)).astype(f8))
    wp8, wp8l = pack_w(f["lnf_g"][:, None] * f["Wp"] * SP32,
                       f["lnf_b"] @ f["Wp"] + f["bp"], KT, V, lo=True)
    pe = _positional_encoding(NTOK, D)

    shared = {
        "eye32": np.eye(128, dtype=np.float32),
        "wemb8": wemb8, "wembl": wembl, "wqkvo8": qkvo, "w18": w1p_,
        "w28": w2p_, "wp8": wp8, "wp8l": wp8l,
    }
    in_maps = []
    for c in range(NCORES):
        bb, hh = c // 2, c % 2
        n0 = hh * T
        m = dict(shared)
        xt = x[bb, n0:n0 + T, :].T.reshape(VP, 128, T).transpose(1, 0, 2) \
            .reshape(128, VP * T)
        x8 = np.ascontiguousarray(xt.astype(f8))
        m["xT8"] = x8
        m["xT8l"] = np.ascontiguousarray(
            (xt - x8.astype(np.float32)).astype(f8))
        m["peb"] = np.ascontiguousarray(pe[n0:n0 + T] + bemb)
        m["slotoff"] = np.array([[(1 - hh) * CCSZ]], np.int32)
        in_maps.append(m)
    return in_maps


_NC_CACHE = []


def kernel(**inputs):
    import time
    from concourse.bass_utils import run_bass_kernel_spmd

    in_maps = _prep_inputs(inputs)
    if not _NC_CACHE:
        _NC_CACHE.append(build_nc())
    nc = _NC_CACHE[0]
    t0 = time.time()
    res = run_bass_kernel_spmd(nc, in_maps, core_ids=list(range(NCORES)))
    t1 = time.time()
    print(f"[kernel] run_bass_kernel_spmd wall: {(t1 - t0) * 1e3:.1f} ms",
          file=sys.stderr)
    out = np.empty((B, NTOK, V), np.float32)
    for c in range(NCORES):
        out[c // 2, (c % 2) * T:(c % 2) * T + T, :] = \
            res.results[c]["logits16"].astype(np.float32)
    return out
